# revision 1
# baseline (speedup 1.0000x reference)
"""Trainium2 Bass kernel for nn_Baseline_SelfGCN (gnn_message_passing).

Data-parallel over batch: 8 NeuronCores x 8 images each. Each core:
  - GAP of x_global + BN(gb)                          -> out[:, 0:2048]
  - mask downsample -> onehot -> counts/drop logic
  - segment-sum pooling of x_gcn (PE transpose + onehot matmul); the
    1/count scaling is applied after the (linear) layer-1 matmul
  - 2-layer GCN (x@W -> blockdiag(adjT)@s + bias -> BN -> relu); the self
    branch shares layer-1 x@W1 (self_feat = mask_feat with the dropped
    part's row zeroed, so s_self = rowmask * s)
  - mean over parts + BN(gn)                          -> out[:, 2048:6144]
  - concat features                                   -> out[:, 6144:43008]

Large matmuls run in float32r (full-rate PE; inputs rounded at producing
ops / cast-DMA). Mean/counts/drop-logic matmuls and transposes stay fp32.

Self-contained: hardcodes shapes; host side only shards/gathers.
"""

import numpy as np

import concourse.bass as bass
import concourse.mybir as mybir
import concourse.tile as tile
from concourse.masks import make_identity

F32 = mybir.dt.float32
F32R = mybir.dt.float32r
I32 = mybir.dt.int32
AF = mybir.ActivationFunctionType
OP = mybir.AluOpType

BL = 8          # images per core
C = 2048
HW = 256        # Hf*Wf
NPARTS = 9      # graph nodes (parts 1..9)
R = BL * NPARTS  # 72 rows = (image, part)
EPS = 1e-5
NCH = 4         # 2048 / 512 N-chunks
KT = 16         # 2048 / 128 K-tiles
OUTW = 3 * C + 2 * NPARTS * C  # 43008

MM_FAST = True  # float32r for the big matmuls
DTM = F32R if MM_FAST else F32


def legalize_waits(nc, max_waits=1):
    """Split multi-wait instructions: this walrus build allows only one
    embedded sync-wait per instruction; hoist extras onto standalone
    InstEventSemaphore waits on the same engine."""
    cnt = 0
    for fn in nc.m.functions:
        for blk in fn.blocks:
            out = []
            for inst in blk.instructions:
                si = inst.sync_info
                if si is not None and si.on_wait and len(si.on_wait) > max_waits:
                    waits = list(si.on_wait)
                    for w in waits[:-max_waits]:
                        cnt += 1
                        wi = mybir.InstEventSemaphore(
                            name=f"wsplit{cnt}_{inst.name}", ins=[], outs=[],
                            sync_info=mybir.SyncInfo(on_wait=[w], on_update=[]))
                        wi.engine = inst.engine
                        nc.register_instruction(wi)
                        out.append(wi)
                    si.on_wait = waits[-max_waits:]
                    inst.sync_info = si
                out.append(inst)
            blk.instructions = out
    return cnt


def _bcast_part(ap, n):
    """Broadcast a DRAM AP across n leading partitions (step-0 dim)."""
    return bass.AP(tensor=ap.tensor, offset=ap.offset, ap=[[0, n]] + list(ap.ap))


def build_bass():
    nc = bass.Bass()

    xg_p = nc.declare_dram_parameter("x_global", [BL, C, HW], F32, isOutput=False)
    xc_p = nc.declare_dram_parameter("x_gcn", [BL, C, HW], F32, isOutput=False)
    mk_p = nc.declare_dram_parameter("mask", [BL, 256, 256], I32, isOutput=False)
    adj_p = nc.declare_dram_parameter("adj", [BL, NPARTS, NPARTS], F32, isOutput=False)
    w1_p = nc.declare_dram_parameter("W1", [C, C], F32, isOutput=False)
    w2_p = nc.declare_dram_parameter("W2", [C, C], F32, isOutput=False)
    b1_p = nc.declare_dram_parameter("b1", [C], F32, isOutput=False)
    b2_p = nc.declare_dram_parameter("b2", [C], F32, isOutput=False)
    bn1 = {k: nc.declare_dram_parameter(k + "1", [NPARTS * C], F32, isOutput=False)
           for k in ("g", "be", "rm", "rv")}
    bn2 = {k: nc.declare_dram_parameter(k + "2", [NPARTS * C], F32, isOutput=False)
           for k in ("g", "be", "rm", "rv")}
    gb = {k: nc.declare_dram_parameter("gb_" + k, [C], F32, isOutput=False)
          for k in ("g", "b", "rm", "rv")}
    gn = {k: nc.declare_dram_parameter("gn_" + k, [C], F32, isOutput=False)
          for k in ("g", "b", "rm", "rv")}
    out_p = nc.declare_dram_parameter("out", [BL, OUTW], F32, isOutput=True)

    with tile.TileContext(nc) as tc:
        with (
            tc.tile_pool(name="consts", bufs=1) as cs,
            tc.tile_pool(name="ps", bufs=8, space="PSUM") as ps,
            tc.tile_pool(name="dram", bufs=1, space="DRAM") as dp,
        ):
            # ---------------- constants ----------------
            ident = cs.tile([128, 128], F32)
            make_identity(nc, ident[:])

            iota_i = cs.tile([128, NPARTS], I32)
            nc.gpsimd.iota(iota_i[:], pattern=[[1, NPARTS]], base=1, channel_multiplier=0)
            iota_f = cs.tile([128, NPARTS], F32)
            nc.vector.tensor_copy(out=iota_f[:], in_=iota_i[:])

            ones_col = cs.tile([128, 1], F32)
            nc.vector.memset(ones_col[:], 1.0)

            # strictly-lower-triangular L9: L[q,p] = 1 if q < p
            L9 = cs.tile([NPARTS, NPARTS], F32)
            nc.gpsimd.memset(L9[:], 0.0)
            nc.gpsimd.affine_select(
                out=L9[:], in_=L9[:], compare_op=OP.is_ge, fill=1.0,
                base=0, pattern=[[-1, NPARTS]], channel_multiplier=1,
            )

            # block "mean over parts" matrix (72, 8): 1/9 on image blocks
            # band-select: onesblk[r, b] = 1/9 iff 0 <= r - 9b <= 8
            onesblk = cs.tile([R, BL], F32)
            nc.gpsimd.memset(onesblk[:], 1.0 / NPARTS)
            nc.gpsimd.affine_select(
                out=onesblk[:], in_=onesblk[:], compare_op=OP.is_ge, fill=0.0,
                base=0, pattern=[[-NPARTS, BL]], channel_multiplier=1)
            nc.gpsimd.affine_select(
                out=onesblk[:], in_=onesblk[:], compare_op=OP.is_ge, fill=0.0,
                base=NPARTS - 1, pattern=[[NPARTS, BL]], channel_multiplier=-1)

            sgn = cs.tile([BL, C], F32)
            tgn = cs.tile([BL, C], F32)
            sgb = cs.tile([128, KT], F32)
            tgb = cs.tile([128, KT], F32)

            # DRAM scratch
            scr = {li: (dp.tile([NPARTS, C], F32, name=f"scr_s{li}"),
                        dp.tile([NPARTS, C], F32, name=f"scr_t{li}"))
                   for li in (0, 1)}
            scr_sm = dp.tile([NPARTS, BL], F32, name="scr_sm")
            scr_rc = dp.tile([NPARTS, BL], F32, name="scr_rc")

            # ---------------- BN folds (scoped scratch pool) ----------------
            with tc.tile_pool(name="foldtmp", bufs=1) as ftp:
                # per-layer folds: s = g/sqrt(rv+eps); t = (b_lin - rm)*s + be
                for li, (bnp, blin) in enumerate(((bn1, b1_p), (bn2, b2_p))):
                    st = ftp.tile([NPARTS, C], F32, tag="f_s", name=f"st{li}")
                    gt = ftp.tile([NPARTS, C], F32, tag="f_g", name=f"gt{li}")
                    tt = ftp.tile([NPARTS, C], F32, tag="f_t", name=f"tt{li}")
                    rmt = ftp.tile([NPARTS, C], F32, tag="f_rm", name=f"rmt{li}")
                    bet = ftp.tile([NPARTS, C], F32, tag="f_be", name=f"bet{li}")
                    nc.sync.dma_start(out=st[:], in_=bnp["rv"].rearrange("(p d) -> p d", d=C))
                    nc.sync.dma_start(out=gt[:], in_=bnp["g"].rearrange("(p d) -> p d", d=C))
                    nc.sync.dma_start(out=rmt[:], in_=bnp["rm"].rearrange("(p d) -> p d", d=C))
                    nc.sync.dma_start(out=bet[:], in_=bnp["be"].rearrange("(p d) -> p d", d=C))
                    nc.sync.dma_start(out=tt[:], in_=blin[None, :].to_broadcast([NPARTS, C]))
                    nc.vector.tensor_scalar_add(st[:], st[:], EPS)
                    nc.scalar.activation(out=st[:], in_=st[:], func=AF.Sqrt)
                    nc.vector.reciprocal(out=st[:], in_=st[:])
                    nc.vector.tensor_mul(st[:], st[:], gt[:])
                    nc.vector.tensor_sub(tt[:], tt[:], rmt[:])
                    nc.vector.tensor_mul(tt[:], tt[:], st[:])
                    nc.vector.tensor_add(tt[:], tt[:], bet[:])
                    nc.sync.dma_start(out=scr[li][0][:], in_=st[:])
                    nc.sync.dma_start(out=scr[li][1][:], in_=tt[:])

                # gn fold at (8, 2048)
                gnw = {}
                for k in ("g", "b", "rm", "rv"):
                    t = ftp.tile([BL, C], F32, tag=f"gn_{k}", name=f"gnl_{k}")
                    nc.sync.dma_start(out=t[:], in_=gn[k][None, :].to_broadcast([BL, C]))
                    gnw[k] = t
                nc.vector.tensor_scalar_add(sgn[:], gnw["rv"][:], EPS)
                nc.scalar.activation(out=sgn[:], in_=sgn[:], func=AF.Sqrt)
                nc.vector.reciprocal(out=sgn[:], in_=sgn[:])
                nc.vector.tensor_mul(sgn[:], sgn[:], gnw["g"][:])
                nc.vector.tensor_mul(tgn[:], gnw["rm"][:], sgn[:])
                nc.vector.tensor_sub(tgn[:], gnw["b"][:], tgn[:])

                # gb fold at (128, 16), c-minor layout: c = j*128 + p
                gbw = {}
                for k in ("g", "b", "rm", "rv"):
                    t = ftp.tile([128, KT], F32, tag=f"gb_{k}", name=f"gbl_{k}")
                    src = gb[k][:]
                    nc.sync.dma_start(
                        out=t[:],
                        in_=bass.AP(tensor=src.tensor, offset=src.offset,
                                    ap=[[1, 128], [128, KT]]))
                    gbw[k] = t
                nc.vector.tensor_scalar_add(sgb[:], gbw["rv"][:], EPS)
                nc.scalar.activation(out=sgb[:], in_=sgb[:], func=AF.Sqrt)
                nc.vector.reciprocal(out=sgb[:], in_=sgb[:])
                nc.vector.tensor_mul(sgb[:], sgb[:], gbw["g"][:])
                nc.vector.tensor_mul(tgb[:], gbw["rm"][:], sgb[:])
                nc.vector.tensor_sub(tgb[:], gbw["b"][:], tgb[:])
                nc.vector.tensor_scalar_mul(sgb[:], sgb[:], 1.0 / HW)  # GAP 1/HW

            # block-diag adj^T (72, 72)
            BD = cs.tile([R, R], F32)
            nc.gpsimd.memset(BD[:], 0.0)
            for b in range(BL):
                nc.sync.dma_start(
                    out=BD[NPARTS * b:NPARTS * (b + 1), NPARTS * b:NPARTS * (b + 1)],
                    in_=adj_p[b].transpose([1, 0]))
            BDr = cs.tile([R, R], DTM)
            nc.vector.tensor_copy(out=BDr[:], in_=BD[:])

            selfm98 = cs.tile([NPARTS, BL], F32)   # keep-mask per (part, image)
            rec98 = cs.tile([NPARTS, BL], F32)     # 1/(count+eps) per (part, image)
            mrec72 = cs.tile([R, 1], F32)          # mask branch row scale
            srec72 = cs.tile([R, 1], F32)          # self branch row scale
            G = cs.tile([128, BL, KT], F32)        # GAP sums, free=(b, j)
            Gn = cs.tile([128, BL, KT], F32)
            GT = cs.tile([128, 128], F32)

            with (
                tc.tile_pool(name="stream", bufs=3) as stream,
                tc.tile_pool(name="xtp", bufs=2) as xtp,
                tc.tile_pool(name="wp", bufs=5) as wp,
                tc.tile_pool(name="mm", bufs=1) as mm,
                tc.tile_pool(name="small", bufs=2) as sp,
            ):
                mfT = mm.tile([128, KT, R], DTM, tag="e")  # layer-1 lhsT (raw sums)

                # ---------------- per-image: mask, pooling, GAP ----------------
                for b in range(BL):
                    # mask rows -> (128,2) tile (partition = h*16+w, col = h-half)
                    msrc = mk_p[b, ::16, ::16]  # (16,16)
                    mr = sp.tile([128, 2], I32, tag="mr", name=f"mr{b}")
                    nc.sync.dma_start(out=mr[:, 0:1], in_=msrc[0:8, :])
                    nc.sync.dma_start(out=mr[:, 1:2], in_=msrc[8:16, :])
                    mrf = sp.tile([128, 2], F32, tag="mrf", name=f"mrf{b}")
                    nc.vector.tensor_copy(out=mrf[:], in_=mr[:])
                    oh = sp.tile([128, 2, NPARTS], DTM, tag="oh", name=f"oh{b}")
                    for h in range(2):
                        nc.vector.tensor_scalar(
                            out=oh[:, h, :], in0=iota_f[:], scalar1=mrf[:, h:h + 1],
                            scalar2=None, op0=OP.is_equal)

                    # counts -> rec / present / first-present -> selfmask
                    pcnt = ps.tile([NPARTS, 1], F32, tag="ps", name=f"pcnt{b}")
                    for h in range(2):
                        nc.tensor.matmul(pcnt[:], oh[:, h, :].bitcast(F32),
                                         ones_col[:],
                                         start=(h == 0), stop=(h == 1))
                    nc.vector.tensor_scalar_add(rec98[:, b:b + 1], pcnt[:], 1e-8)
                    nc.vector.reciprocal(out=rec98[:, b:b + 1], in_=rec98[:, b:b + 1])
                    pres = sp.tile([NPARTS, 1], F32, tag="pres", name=f"pres{b}")
                    nc.vector.tensor_scalar(out=pres[:], in0=pcnt[:], scalar1=0.0,
                                            scalar2=None, op0=OP.is_gt)
                    ppre = ps.tile([NPARTS, 1], F32, tag="ps", name=f"ppre{b}")
                    nc.tensor.matmul(ppre[:], L9[:], pres[:], start=True, stop=True)
                    isz = sp.tile([NPARTS, 1], F32, tag="isz", name=f"isz{b}")
                    nc.vector.tensor_scalar(out=isz[:], in0=ppre[:], scalar1=0.0,
                                            scalar2=None, op0=OP.is_equal)
                    nc.vector.tensor_mul(isz[:], isz[:], pres[:])
                    nc.vector.tensor_scalar(out=selfm98[:, b:b + 1], in0=isz[:],
                                            scalar1=-1.0, scalar2=1.0,
                                            op0=OP.mult, op1=OP.add)

                    # x_gcn load (c-minor: partition p = c%128, j = c//128)
                    xct = stream.tile([128, KT, HW], F32, tag="stream", name=f"xct{b}")
                    nc.sync.dma_start(
                        out=xct[:],
                        in_=xc_p[b].rearrange("(j p) hw -> p j hw", p=128))
                    # transpose to (hw, c) via PE
                    xt = xtp.tile([128, 2, C], DTM, tag="xt", name=f"xt{b}")
                    for j in range(KT):
                        for h in range(2):
                            ptt = ps.tile([128, 128], F32, tag="ps",
                                          name=f"ptt{b}_{j}_{h}")
                            nc.tensor.transpose(ptt[:], xct[:, j, 128 * h:128 * (h + 1)],
                                                ident[:])
                            nc.any.tensor_copy(out=xt[:, h, 128 * j:128 * (j + 1)],
                                               in_=ptt[:])
                    # segment raw sums: (9, 2048) = oh.T @ xt
                    mfeat = sp.tile([NPARTS, C], F32, tag="mfeat", name=f"mfeat{b}", bufs=1)
                    for n in range(NCH):
                        pseg = ps.tile([NPARTS, 512], F32, tag="ps", name=f"pseg{b}_{n}")
                        for h in range(2):
                            nc.tensor.matmul(pseg[:], oh[:, h, :],
                                             xt[:, h, 512 * n:512 * (n + 1)],
                                             start=(h == 0), stop=(h == 1))
                        nc.any.tensor_copy(out=mfeat[:, 512 * n:512 * (n + 1)],
                                           in_=pseg[:])
                    # transpose raw sums into layer-1 lhsT layout
                    for kt in range(KT):
                        ptm = ps.tile([128, NPARTS], F32, tag="ps", name=f"ptm{b}_{kt}")
                        nc.tensor.transpose(ptm[:], mfeat[:, 128 * kt:128 * (kt + 1)],
                                            ident[0:NPARTS, 0:NPARTS])
                        nc.any.tensor_copy(
                            out=mfT[:, kt, NPARTS * b:NPARTS * (b + 1)], in_=ptm[:])

                    # x_global load + GAP reduce
                    xgt = stream.tile([128, KT, HW], F32, tag="stream", name=f"xgt{b}")
                    nc.sync.dma_start(
                        out=xgt[:],
                        in_=xg_p[b].rearrange("(j p) hw -> p j hw", p=128))
                    nc.vector.reduce_sum(out=G[:, b, :], in_=xgt[:],
                                         axis=mybir.AxisListType.X)

                # row scales via DRAM bounce (transposed read to (72,1))
                nc.sync.dma_start(out=scr_sm[:], in_=selfm98[:])
                nc.sync.dma_start(out=scr_rc[:], in_=rec98[:])
                smv, rcv = scr_sm[:], scr_rc[:]
                nc.sync.dma_start(
                    out=mrec72[:],
                    in_=bass.AP(tensor=rcv.tensor, offset=rcv.offset,
                                ap=[[1, BL], [BL, NPARTS], [0, 1]]))
                nc.sync.dma_start(
                    out=srec72[:],
                    in_=bass.AP(tensor=smv.tensor, offset=smv.offset,
                                ap=[[1, BL], [BL, NPARTS], [0, 1]]))
                nc.vector.tensor_mul(srec72[:], srec72[:], mrec72[:])

                # ---------------- bnfeat_global output ----------------
                nc.vector.tensor_tensor(
                    Gn[:], G[:],
                    sgb[:, None, :].to_broadcast([128, BL, KT]), OP.mult)
                nc.vector.tensor_tensor(
                    Gn[:], Gn[:],
                    tgb[:, None, :].to_broadcast([128, BL, KT]), OP.add)
                pG = ps.tile([128, 128], F32, tag="ps")
                nc.tensor.transpose(pG[:], Gn[:].rearrange("p b j -> p (b j)"), ident[:])
                nc.any.tensor_copy(out=GT[:], in_=pG[:])
                nc.sync.dma_start(
                    out=out_p[:, 0:C].rearrange("b (j p) -> b j p", p=128), in_=GT[:])

                # layer-1 BN reps (72, 2048) via partition-broadcast reload
                srep = cs.tile([R, C], F32, tag="srep", name="srep1")
                trep = cs.tile([R, C], F32, tag="trep", name="trep1")
                nc.sync.dma_start(out=srep[:], in_=_bcast_part(scr[0][0][:], BL))
                nc.sync.dma_start(out=trep[:], in_=_bcast_part(scr[0][1][:], BL))

                # ---------------- GCN layer 1 ----------------
                s_all = mm.tile([R, C], DTM, tag="a")
                s_self = mm.tile([R, C], DTM, tag="b")
                psl1 = [ps.tile([R, 512], F32, tag="ps", name=f"psl1_{i}")
                        for i in range(NCH)]
                for ha in range(2):
                    for kt in range(KT):
                        w = wp.tile([128, C // 2], DTM, tag="w", name=f"w1_{ha}_{kt}")
                        if MM_FAST:  # SWDGE cast-DMA rounds f32 -> f32r in flight
                            nc.gpsimd.dma_start(
                                out=w[:],
                                in_=w1_p[128 * kt:128 * (kt + 1),
                                         1024 * ha:1024 * (ha + 1)])
                        else:
                            nc.sync.dma_start(
                                out=w[:],
                                in_=w1_p[128 * kt:128 * (kt + 1),
                                         1024 * ha:1024 * (ha + 1)])
                        for i in range(2):
                            n = 2 * ha + i
                            nc.tensor.matmul(psl1[n][:], mfT[:, kt, :],
                                             w[:, 512 * i:512 * (i + 1)],
                                             start=(kt == 0), stop=(kt == KT - 1))
                    for i in range(2):
                        n = 2 * ha + i
                        nc.vector.tensor_scalar(
                            out=s_all[:, 512 * n:512 * (n + 1)], in0=psl1[n][:],
                            scalar1=mrec72[:, 0:1], scalar2=None, op0=OP.mult)
                        nc.vector.tensor_scalar(
                            out=s_self[:, 512 * n:512 * (n + 1)], in0=psl1[n][:],
                            scalar1=srec72[:, 0:1], scalar2=None, op0=OP.mult)

                # bmm + BN1 + relu for both branches
                x1 = {}
                for br, s_in, xtag in (("m", s_all, "c"), ("s", s_self, "d")):
                    xo = mm.tile([R, C], F32, tag=xtag, name=f"x1{br}")
                    for n in range(NCH):
                        po = ps.tile([R, 512], F32, tag="ps", name=f"po1{br}{n}")
                        nc.tensor.matmul(po[:], BDr[:],
                                         s_in[:, 512 * n:512 * (n + 1)],
                                         start=True, stop=True)
                        sl = slice(512 * n, 512 * (n + 1))
                        nc.vector.tensor_tensor(xo[:, sl], po[:], srep[:, sl], OP.mult)
                        nc.vector.tensor_tensor(xo[:, sl], xo[:, sl], trep[:, sl], OP.add)
                        nc.scalar.activation(out=xo[:, sl], in_=xo[:, sl], func=AF.Relu)
                    x1[br] = xo

                # transpose x1 for layer 2
                x1T = {}
                for br, ttag in (("m", "e"), ("s", "f")):
                    xt1 = mm.tile([128, KT, R], DTM, tag=ttag, name=f"x1T{br}")
                    for kt in range(KT):
                        pt1 = ps.tile([128, R], F32, tag="ps", name=f"pt1{br}{kt}")
                        nc.tensor.transpose(pt1[:], x1[br][:, 128 * kt:128 * (kt + 1)],
                                            ident[0:R, 0:R])
                        nc.any.tensor_copy(out=xt1[:, kt, :], in_=pt1[:])
                    x1T[br] = xt1

                # ---------------- GCN layer 2 ----------------
                psl2 = {br: [ps.tile([R, 512], F32, tag="ps", name=f"psl2_{br}_{i}")
                             for i in range(NCH)] for br in ("m", "s")}
                for kt in range(KT):
                    w = wp.tile([128, C], DTM, tag="w", name=f"w2_{kt}")
                    if MM_FAST:
                        nc.sync.dma_start(out=w[:],
                                          in_=w2_p[128 * kt:128 * (kt + 1), :].bitcast(F32R))
                        nc.vector.tensor_copy(out=w[:], in_=w[:].bitcast(F32))
                    else:
                        nc.sync.dma_start(out=w[:], in_=w2_p[128 * kt:128 * (kt + 1), :])
                    for br in ("m", "s"):
                        for n in range(NCH):
                            nc.tensor.matmul(psl2[br][n][:], x1T[br][:, kt, :],
                                             w[:, 512 * n:512 * (n + 1)],
                                             start=(kt == 0), stop=(kt == KT - 1))
                s2 = {}
                for br, stag in (("m", "a"), ("s", "b")):
                    t = mm.tile([R, C], DTM, tag=stag, name=f"s2{br}")
                    for n in range(NCH):
                        nc.any.tensor_copy(out=t[:, 512 * n:512 * (n + 1)],
                                           in_=psl2[br][n][:])
                    s2[br] = t

                # layer-2 BN reps (reuse slots)
                srep2 = cs.tile([R, C], F32, tag="srep", name="srep2")
                trep2 = cs.tile([R, C], F32, tag="trep", name="trep2")
                nc.sync.dma_start(out=srep2[:], in_=_bcast_part(scr[1][0][:], BL))
                nc.sync.dma_start(out=trep2[:], in_=_bcast_part(scr[1][1][:], BL))

                # bmm + BN2 + relu -> x2 ; outputs
                cat_off = {"m": 3 * C, "s": 3 * C + NPARTS * C}
                bnf_off = {"m": C, "s": 2 * C}
                for br, xtag in (("m", "c"), ("s", "d")):
                    x2 = mm.tile([R, C], F32, tag=xtag, name=f"x2{br}")
                    boff = bnf_off[br]
                    off = cat_off[br]
                    catv = out_p[:, off:off + NPARTS * C].rearrange(
                        "b (p d) -> b p d", d=C)
                    # full-width mean staging tile (reuses the dead mfeat slot)
                    bnf = sp.tile([NPARTS, C], F32, tag="mfeat",
                                  name=f"bnf{br}", bufs=1)
                    for n in range(NCH):
                        po = ps.tile([R, 512], F32, tag="ps", name=f"po2{br}{n}")
                        nc.tensor.matmul(po[:], BDr[:],
                                         s2[br][:, 512 * n:512 * (n + 1)],
                                         start=True, stop=True)
                        sl = slice(512 * n, 512 * (n + 1))
                        nc.vector.tensor_tensor(x2[:, sl], po[:], srep2[:, sl], OP.mult)
                        nc.vector.tensor_tensor(x2[:, sl], x2[:, sl], trep2[:, sl], OP.add)
                        nc.scalar.activation(out=x2[:, sl], in_=x2[:, sl], func=AF.Relu)
                        # cat output chunk (drains while later chunks compute)
                        nc.sync.dma_start(out=catv[:, :, sl], in_=x2[:, sl])
                        # mean over parts + BN(gn) (exact fp32)
                        pf = ps.tile([BL, 512], F32, tag="ps", name=f"pf{br}{n}")
                        nc.tensor.matmul(pf[:], onesblk[:], x2[:, sl],
                                         start=True, stop=True)
                        nc.vector.tensor_tensor(bnf[0:BL, sl], pf[:], sgn[:, sl],
                                                OP.mult)
                        nc.vector.tensor_tensor(bnf[0:BL, sl], bnf[0:BL, sl],
                                                tgn[:, sl], OP.add)
                    nc.sync.dma_start(out=out_p[:, boff:boff + C], in_=bnf[0:BL, :])

    legalize_waits(nc)
    return nc


_CACHE = {}


def kernel(_run_kwargs=None, **inputs):
    run_kwargs = _run_kwargs or {}
    if "nc" not in _CACHE:
        _CACHE["nc"] = build_bass()
    nc = _CACHE["nc"]

    B = inputs["x_global"].shape[0]
    n_cores = 8
    bl = B // n_cores

    rep_names = ["W1", "W2", "b1", "b2", "g1", "be1", "rm1", "rv1",
                 "g2", "be2", "rm2", "rv2",
                 "gb_g", "gb_b", "gb_rm", "gb_rv",
                 "gn_g", "gn_b", "gn_rm", "gn_rv"]

    in_maps = []
    for c in range(n_cores):
        sl = slice(c * bl, (c + 1) * bl)
        m = {
            "x_global": np.ascontiguousarray(
                inputs["x_global"][sl]).reshape(bl, C, HW).astype(np.float32),
            "x_gcn": np.ascontiguousarray(
                inputs["x_gcn"][sl]).reshape(bl, C, HW).astype(np.float32),
            "mask": np.ascontiguousarray(
                inputs["mask"][sl, 0]).astype(np.int32),
            "adj": np.ascontiguousarray(inputs["adj"][sl]).astype(np.float32),
        }
        for k in rep_names:
            m[k] = np.ascontiguousarray(inputs[k]).astype(np.float32)
        in_maps.append(m)

    from concourse.bass_utils import run_bass_kernel_spmd
    res = run_bass_kernel_spmd(nc, in_maps, list(range(n_cores)), **run_kwargs)
    out = np.concatenate([res.results[c]["out"] for c in range(n_cores)], axis=0)
    _CACHE["last_results"] = res
    return out

